# revision 1
# baseline (speedup 1.0000x reference)
"""Trainium2 Bass kernel for nn_CustomNetwork_37031208026716.

Network: 32 layers of (depth-1 butterfly rotation + interleave permutation +
smooth-bend activation y = u + cc*sqrt(u^2 + ik)) on X[65536, 512] fp32.

Strategy ("pair-compose", fp16 resident):
  * Pure data parallel over 8 cores (batch split, 8192 rows/core).
  * Width (512) on partitions as 4 tiles of 128; batch on the free axis,
    fp16 in SBUF.  Interleave permutation via conjugated coordinates:
    physical w at layer l is logical pi_l(w); butterfly pairs w with
    w^delta_l, delta_l = 2^((8-l)%9) (cross-tile when delta>=128).
  * Layer state is the PAIR (U, T): U = u (pre-bend affine value),
    T = sqrt(u^2+ik).  y = U + cc*T is only materialized (as Y, carrying a
    host-tracked additive offset) on layers feeding a cross-tile butterfly,
    so EVERY layer runs exactly two 128x128 fp16 matmul matrices:
      within-tile :  n = MU@U + MT@T      (MT folds prev layer's cc)
      cross-tile  :  n = D_s*Y_g + D_p*Y_g^  (diagonal matrices)
  * Remaining per-layer elementwise work, balanced across DVE/ACT/GpSimd:
      pair layers:  pull U' = n - pb  (DVE tensor_scalar from PSUM; ~6% on
                    ACT Identity), q = U'*U' (75% GpSimd tensor_tensor, 25%
                    DVE fp16 2x), T' = sqrt(q + ik) (ACT, per-partition ik).
      ymat layers:  q = Square(n - pb) directly from PSUM (ACT),
                    T' = sqrt(q + ik) (ACT), Y' = cc*T' + n (DVE stt from
                    PSUM; carries offset pb which the host folds into the
                    next layer's biases).
  * Stationary matrices in bf16 (PE cost is set by the fp16 moving operand;
    bf16 weights keep rel-err ~1e-2, well under the 2e-2 gate).
  * Four 1024-column chunks interleaved per pass to keep ~16 tile-streams
    in flight (hides the cross-engine mm->pull->square->sqrt latency).
  * Host casts fp32->fp16 for input/output transfers (halves HBM traffic).
"""

import numpy as np

BATCH = 65536
W = 512
HALF = 256
DEPTH = 32
NBITS = 9
NCORES = 8
NB = BATCH // NCORES          # batch rows per core
CH = 1024                     # batch columns per on-chip chunk
NTILE = 4                     # width tiles of 128 partitions
MMH = 512                     # moving free-dim per matmul (ISA cap)

CROSS = frozenset(l for l, d in enumerate(
    [1 << ((8 - l) % NBITS) for l in range(DEPTH)]) if d >= 128)
YMAT = frozenset(l for l in range(DEPTH) if (l + 1) in CROSS)

_P_ARR = np.array([(w >> 1) | ((w & 1) << 8) for w in range(W)], dtype=np.int64)


def _invert(p):
    inv = np.empty_like(p)
    inv[p] = np.arange(len(p))
    return inv


def _build_perms():
    pinv = _invert(_P_ARR)
    pis = [np.arange(W)]
    for l in range(DEPTH):
        pis.append(pinv[pis[l]])
    return pis


def _deltas():
    return [1 << ((8 - l) % NBITS) for l in range(DEPTH)]


def host_precompute(thetas, biases, slopes1, slopes2, curvatures):
    pis = _build_perms()
    thetas = thetas.astype(np.float64)
    c_all = np.cos(thetas)
    s_all = np.sin(thetas)
    m1 = np.exp(slopes1.astype(np.float64))
    m2 = np.exp(slopes2.astype(np.float64))
    a_all = (m1 + m2) * 0.5
    cc_all = (m2 - m1) / (2.0 * a_all)
    b_all = np.sinh(biases.astype(np.float64))
    ik_all = np.exp(-curvatures.astype(np.float64))

    idx_w = np.arange(W)
    layers = []
    ofs = np.zeros(W)                 # additive offset carried by Y tiles
    for l in range(DEPTH):
        pi_l, pi_n = pis[l], pis[l + 1]
        delta = 1 << ((8 - l) % NBITS)
        V = pi_l
        idx = V % HALF
        cA = c_all[l][idx]
        sA = np.where(V < HALF, s_all[l][idx], -s_all[l][idx])
        a_eff = a_all[l][pi_n]
        cAp = cA * a_eff
        sAp = sA * a_eff
        bb = b_all[l][pi_n]
        # effective pre-activation bias: subtract what the (offset) inputs
        # injected plus this layer's own bias
        pb = bb + cAp * ofs + sAp * ofs[idx_w ^ delta]
        ent = dict(delta=delta, cAp=cAp, sAp=sAp, pb=pb,
                   cc=cc_all[l][pi_n], ik=ik_all[l][pi_n])
        ofs = pb if l in YMAT else np.zeros(W)
        layers.append(ent)
    return layers, _invert(pis[DEPTH])


# ---- packing ---------------------------------------------------------------

def _wts_layout():
    out = []
    for l in range(DEPTH):
        for g in range(NTILE):
            if l in CROSS:
                out.append((l, g, "Ds"))
                out.append((l, g, "Dp"))
            else:
                out.append((l, g, "MU"))
                out.append((l, g, "MT"))
    return out


def pack_weights(layers):
    import ml_dtypes
    lay = _wts_layout()
    wts = np.zeros((128, len(lay) * 128), dtype=ml_dtypes.bfloat16)
    p128 = np.arange(128)
    for j, (l, g, kind) in enumerate(lay):
        ent = layers[l]
        delta = ent["delta"]
        w = g * 128 + p128
        part = w ^ delta
        A = np.zeros((128, 128), dtype=np.float64)
        if kind == "Ds":
            A[p128, p128] = ent["cAp"][w]
        elif kind == "Dp":
            A[p128, p128] = ent["sAp"][w]
        elif kind == "MU":
            A[p128, p128] = ent["cAp"][w]
            A[p128, p128 ^ delta] = ent["sAp"][w]
        elif kind == "MT":
            prev = layers[l - 1]
            A[p128, p128] = ent["cAp"][w] * prev["cc"][w]
            A[p128, p128 ^ delta] = ent["sAp"][w] * prev["cc"][part]
        wts[:, j * 128:(j + 1) * 128] = A.T.astype(wts.dtype)
    return wts


_PKINDS = ("pb", "npb", "ik", "cc")
PRM_COLS = len(_PKINDS) * DEPTH * NTILE


def pack_params(layers):
    prm = np.zeros((128, PRM_COLS), dtype=np.float32)
    for l, ent in enumerate(layers):
        vals = dict(pb=ent["pb"], npb=-ent["pb"], ik=ent["ik"],
                    cc=ent["cc"])
        for k, kind in enumerate(_PKINDS):
            v = vals[kind]
            for g in range(NTILE):
                prm[:, (k * DEPTH + l) * NTILE + g] = v[g * 128:(g + 1) * 128]
    return prm


# ---- bass module -----------------------------------------------------------

def build_nc(nb=NB, ch=CH):
    from concourse import bacc, mybir
    from concourse.tile import TileContext

    f32 = mybir.dt.float32
    f16 = mybir.dt.float16
    AT = mybir.ActivationFunctionType
    OP = mybir.AluOpType

    deltas = _deltas()
    lay = _wts_layout()
    widx = {key: j for j, key in enumerate(lay)}

    nc = bacc.Bacc(target_bir_lowering=False, debug=False)
    xt = nc.declare_dram_parameter("xt", [W, nb], f16, isOutput=False)
    prm_d = nc.declare_dram_parameter("prm", [128, PRM_COLS], f32,
                                      isOutput=False)
    wts_d = nc.declare_dram_parameter("wts", [128, len(lay) * 128],
                                      mybir.dt.bfloat16, isOutput=False)
    out_d = nc.declare_dram_parameter("out", [W, nb], f16, isOutput=True)

    nchunk = nb // ch

    with TileContext(nc) as tc:
        with (
            tc.tile_pool(name="const", bufs=1) as cpool,
            tc.tile_pool(name="u", bufs=5) as upool,
            tc.tile_pool(name="t", bufs=5) as tpool,
            tc.tile_pool(name="q", bufs=6) as qpool,
            tc.tile_pool(name="y", bufs=5) as ypool,
            tc.tile_pool(name="ps", bufs=4, space="PSUM") as pspool,
        ):
            prm = cpool.tile([128, PRM_COLS], f32, tag="prm", name="prm")
            nc.sync.dma_start(out=prm, in_=prm_d[:, :])
            wsb = cpool.tile([128, len(lay) * 128], mybir.dt.bfloat16,
                             tag="wts", name="wsb")
            nw = len(lay) * 128
            for s in range(4):
                lo, hi = s * nw // 4, (s + 1) * nw // 4
                nc.sync.dma_start(out=wsb[:, lo:hi], in_=wts_d[:, lo:hi])

            def pcol(kind, l, g):
                k = _PKINDS.index(kind)
                c = (k * DEPTH + l) * NTILE + g
                return prm[:, c:c + 1]

            def lhs(l, g, kind):
                j = widx[(l, g, kind)]
                return wsb[:, j * 128:(j + 1) * 128]

            xt_r = xt.rearrange("(g p) b -> p g b", g=NTILE)
            out_r = out_d.rearrange("(g p) b -> p g b", g=NTILE)

            SUPER = 4
            for cp in range(nchunk // SUPER):
                cs = list(range(cp * SUPER, (cp + 1) * SUPER))
                # layer-0 inputs are Y tiles (offset 0)
                Ys, Us, Ts = {}, {}, {}
                for c in cs:
                    Ys[c] = {}
                    for g in range(NTILE):
                        Ys[c][g] = ypool.tile([128, ch], f16, tag=f"y{g}",
                                              name=f"y{g}")
                        nc.sync.dma_start(
                            out=Ys[c][g],
                            in_=xt_r[:, g, c * ch:(c + 1) * ch])
                    Us[c], Ts[c] = {}, {}

                for l in range(DEPTH):
                  delta = deltas[l]
                  for c in cs:
                    Y, U, T = Ys[c], Us[c], Ts[c]
                    Un, Tn, Yn = {}, {}, {}
                    for g in range(NTILE):
                        ps = pspool.tile([128, ch], f32, tag="ps", name="ps")
                        for h in range(0, ch, MMH):
                            sl = slice(h, h + MMH)
                            if l in CROSS:
                                gp = g ^ (delta >> 7)
                                nc.tensor.matmul(
                                    ps[:, sl], lhs(l, g, "Ds"), Y[g][:, sl],
                                    start=True, stop=False)
                                nc.tensor.matmul(
                                    ps[:, sl], lhs(l, g, "Dp"), Y[gp][:, sl],
                                    start=False, stop=True)
                            else:
                                nc.tensor.matmul(
                                    ps[:, sl], lhs(l, g, "MU"), U[g][:, sl],
                                    start=True, stop=False)
                                nc.tensor.matmul(
                                    ps[:, sl], lhs(l, g, "MT"), T[g][:, sl],
                                    start=False, stop=True)
                        if l in YMAT:
                            # q = (n - pb)^2 straight from PSUM on ACT
                            q = qpool.tile([128, ch], f16, tag="q", name="q")
                            nc.scalar.activation(
                                q, ps, AT.Square, bias=pcol("npb", l, g),
                                scale=1.0)
                            t = tpool.tile([128, ch], f16, tag=f"t{g}",
                                           name=f"t{g}")
                            nc.scalar.activation(
                                t, q, AT.Sqrt, bias=pcol("ik", l, g),
                                scale=1.0)
                            y = ypool.tile([128, ch], f16, tag=f"y{g}",
                                           name=f"y{g}")
                            nc.vector.scalar_tensor_tensor(
                                y, t, pcol("cc", l, g), ps, OP.mult, OP.add)
                            Yn[g] = y
                        else:
                            u = upool.tile([128, ch], f16, tag=f"u{g}",
                                           name=f"u{g}")
                            if (l * NTILE + g + c) % 16 != 0:
                                nc.vector.tensor_scalar(
                                    u, ps, pcol("pb", l, g), None,
                                    OP.subtract)
                            else:
                                nc.scalar.activation(
                                    u, ps, AT.Identity,
                                    bias=pcol("npb", l, g), scale=1.0)
                            q = qpool.tile([128, ch], f16, tag="q", name="q")
                            if (l + g + c) % 4 < 3:  # 75% squares on GpSimd
                                nc.gpsimd.tensor_tensor(q, u, u, OP.mult)
                            else:
                                nc.vector.tensor_tensor(q, u, u, OP.mult)
                            t = tpool.tile([128, ch], f16, tag=f"t{g}",
                                           name=f"t{g}")
                            nc.scalar.activation(
                                t, q, AT.Sqrt, bias=pcol("ik", l, g),
                                scale=1.0)
                            Un[g] = u
                            Tn[g] = t
                    if Yn:
                        Ys[c] = Yn
                    Us[c] = Un or U
                    Ts[c] = Tn or T

                for c in cs:
                    for g in range(NTILE):
                        y = ypool.tile([128, ch], f16, tag=f"y{g}",
                                       name=f"o{g}")
                        nc.vector.scalar_tensor_tensor(
                            y, Ts[c][g], pcol("cc", DEPTH - 1, g), Us[c][g],
                            OP.mult, OP.add)
                        nc.sync.dma_start(
                            out=out_r[:, g, c * ch:(c + 1) * ch], in_=y)
    nc.compile()
    return nc


_NC_CACHE = {}

TRACE = False
TRACE_KWARGS = {}
LAST_RESULTS = None


def _get_nc(nb, ch):
    key = (nb, ch)
    if key not in _NC_CACHE:
        _NC_CACHE[key] = build_nc(nb, ch)
    return _NC_CACHE[key]


def kernel(X, thetas, biases, slopes1, slopes2, curvatures):
    global LAST_RESULTS
    from concourse.bass_utils import run_bass_kernel_spmd

    X = np.asarray(X)
    layers, out_perm = host_precompute(
        np.asarray(thetas), np.asarray(biases), np.asarray(slopes1),
        np.asarray(slopes2), np.asarray(curvatures))
    prm = pack_params(layers)
    wts = pack_weights(layers)

    nc = _get_nc(NB, CH)
    in_maps = []
    for cid in range(NCORES):
        shard = np.ascontiguousarray(
            X[cid * NB:(cid + 1) * NB, :].T.astype(np.float16))
        in_maps.append({"xt": shard, "prm": prm, "wts": wts})

    res = run_bass_kernel_spmd(nc, in_maps, list(range(NCORES)),
                               trace=TRACE, **TRACE_KWARGS)
    LAST_RESULTS = res
    out = np.empty((BATCH, W), dtype=np.float32)
    for cid in range(NCORES):
        o = res.results[cid]["out"]          # [512, NB] fp16 physical order
        out[cid * NB:(cid + 1) * NB, :] = o[out_perm, :].T.astype(np.float32)
    return out



# revision 2
# speedup vs baseline: 1.1893x; 1.1893x over previous
"""Trainium2 Bass kernel for nn_CustomNetwork_37031208026716.

Network: 32 layers of (depth-1 butterfly rotation + interleave permutation +
smooth-bend activation y = u + cc*sqrt(u^2 + ik)) on X[65536, 512] fp32.

Strategy ("normalized (V,T) state", v2):
  * Pure data parallel over 8 cores (batch split, 8192 rows/core).
  * Width (512) on partitions as 4 tiles of 128; batch on the free axis.
    Interleave permutation via conjugated coordinates (physical order w,
    butterfly partner w^delta_l, delta_l = 2^((8-l)%9)).
  * Normalized state pair per layer: V = (n - pb)/sqrt(ik) (PSUM evacuation
    with the affine fold done by ACT Identity scale/bias), T = sqrt(V^2+1).
    y = sqrt(ik)*(V + cc*T) is never materialized: sqrt(ik) and cc fold
    into the next layer's stationary matrices (MV, MT).  Cross-tile layers
    (delta>=128) use 4 diagonal stationaries on (V, T) of the two tiles.
  * Elementwise per layer: ACT evac (Identity, per-partition scale+bias,
    PSUM->SBUF fp16), DVE square (fp16 tensor_tensor, 2x mode), and
    T = sqrt(q+1) either on ACT (Sqrt, bias=1) for ~1/4 of tiles or via a
    3-op DVE bit-trick (s=q+1; int16 >>1; int16 +7643, all 4x mode) for the
    rest.  No GpSimd (it contends with the DVE SBUF port).
  * Chunks of 2048 batch columns: PSUM tiles [128,2048] (4 banks, 2 in
    flight), so ACT/DVE ops run at FD=2048 to amortize fixed overheads.
  * Host folds all activation constants into weights/bias columns in fp64;
    final output = host-side sqrt(ik_31) scale of on-chip F = V + cc*T.
"""

import numpy as np

BATCH = 65536
W = 512
HALF = 256
DEPTH = 32
NBITS = 9
NCORES = 8
NB = BATCH // NCORES          # batch rows per core
CH = 2048                     # batch columns per on-chip chunk (PSUM 4 banks)
NTILE = 4                     # width tiles of 128 partitions
MMH = 512                     # moving free-dim per matmul (ISA cap)
M_MAGIC = 7643                # fp16 magic sqrt constant (max rel err 3.5%)

DELTAS = [1 << ((8 - l) % NBITS) for l in range(DEPTH)]
CROSS = frozenset(l for l, d in enumerate(DELTAS) if d >= 128)

_P_ARR = np.array([(w >> 1) | ((w & 1) << 8) for w in range(W)], dtype=np.int64)


def _invert(p):
    inv = np.empty_like(p)
    inv[p] = np.arange(len(p))
    return inv


def _build_perms():
    pinv = _invert(_P_ARR)
    pis = [np.arange(W)]
    for l in range(DEPTH):
        pis.append(pinv[pis[l]])
    return pis


def host_precompute(thetas, biases, slopes1, slopes2, curvatures):
    pis = _build_perms()
    thetas = thetas.astype(np.float64)
    c_all = np.cos(thetas)
    s_all = np.sin(thetas)
    m1 = np.exp(slopes1.astype(np.float64))
    m2 = np.exp(slopes2.astype(np.float64))
    a_all = (m1 + m2) * 0.5
    cc_all = (m2 - m1) / (2.0 * a_all)
    b_all = np.sinh(biases.astype(np.float64))
    ik_all = np.exp(-curvatures.astype(np.float64))

    layers = []
    for l in range(DEPTH):
        pi_l, pi_n = pis[l], pis[l + 1]
        delta = DELTAS[l]
        V = pi_l
        idx = V % HALF
        cA = c_all[l][idx]
        sA = np.where(V < HALF, s_all[l][idx], -s_all[l][idx])
        a_eff = a_all[l][pi_n]
        ent = dict(
            delta=delta,
            cAp=cA * a_eff,
            sAp=sA * a_eff,
            pb=b_all[l][pi_n],
            cc=cc_all[l][pi_n],
            ik=ik_all[l][pi_n],
        )
        ent["rhat"] = ent["ik"] ** -0.5
        ent["D"] = ent["ik"] ** 0.5
        layers.append(ent)
    return layers, _invert(pis[DEPTH])


# ---- packing ---------------------------------------------------------------

def _wts_layout():
    out = []
    for l in range(DEPTH):
        for g in range(NTILE):
            if l == 0:
                out.append((l, g, "Ds"))
                out.append((l, g, "Dp"))
            elif l in CROSS:
                out.append((l, g, "Dsv"))
                out.append((l, g, "Dpv"))
                out.append((l, g, "Dst"))
                out.append((l, g, "Dpt"))
            else:
                out.append((l, g, "MV"))
                out.append((l, g, "MT"))
    return out


def pack_weights(layers):
    import ml_dtypes
    lay = _wts_layout()
    wts = np.zeros((128, len(lay) * 128), dtype=ml_dtypes.bfloat16)
    p128 = np.arange(128)
    for j, (l, g, kind) in enumerate(lay):
        ent = layers[l]
        delta = ent["delta"]
        w = g * 128 + p128
        wd = w ^ delta          # butterfly partner (physical)
        if l == 0:
            Dp_prev = np.ones(W)
            cc_prev = np.zeros(W)
        else:
            prev = layers[l - 1]
            Dp_prev = prev["D"]
            cc_prev = prev["cc"]
        A = np.zeros((128, 128), dtype=np.float64)
        if kind in ("Ds", "Dsv"):
            A[p128, p128] = ent["cAp"][w] * Dp_prev[w]
        elif kind in ("Dp", "Dpv"):
            A[p128, p128] = ent["sAp"][w] * Dp_prev[wd]
        elif kind == "Dst":
            A[p128, p128] = ent["cAp"][w] * Dp_prev[w] * cc_prev[w]
        elif kind == "Dpt":
            A[p128, p128] = ent["sAp"][w] * Dp_prev[wd] * cc_prev[wd]
        elif kind == "MV":
            A[p128, p128] = ent["cAp"][w] * Dp_prev[w]
            A[p128, p128 ^ delta] = ent["sAp"][w] * Dp_prev[wd]
        elif kind == "MT":
            A[p128, p128] = ent["cAp"][w] * Dp_prev[w] * cc_prev[w]
            A[p128, p128 ^ delta] = ent["sAp"][w] * Dp_prev[wd] * cc_prev[wd]
        wts[:, j * 128:(j + 1) * 128] = A.T.astype(wts.dtype)
    return wts


_PKINDS = ("sR", "bB")
PRM_COLS = len(_PKINDS) * DEPTH * NTILE + NTILE   # + ccF cols


def pack_params(layers):
    prm = np.zeros((128, PRM_COLS), dtype=np.float32)
    for l, ent in enumerate(layers):
        vals = dict(sR=ent["rhat"], bB=-ent["pb"] * ent["rhat"])
        for k, kind in enumerate(_PKINDS):
            v = vals[kind]
            for g in range(NTILE):
                prm[:, (k * DEPTH + l) * NTILE + g] = v[g * 128:(g + 1) * 128]
    ccF = layers[DEPTH - 1]["cc"]
    for g in range(NTILE):
        prm[:, len(_PKINDS) * DEPTH * NTILE + g] = ccF[g * 128:(g + 1) * 128]
    return prm


# ---- bass module -----------------------------------------------------------

def build_nc(nb=NB, ch=CH):
    from concourse import bacc, mybir
    from concourse.tile import TileContext

    f32 = mybir.dt.float32
    f16 = mybir.dt.float16
    i16 = mybir.dt.int16
    bf16 = mybir.dt.bfloat16
    AT = mybir.ActivationFunctionType
    OP = mybir.AluOpType

    lay = _wts_layout()
    widx = {key: j for j, key in enumerate(lay)}

    nc = bacc.Bacc(target_bir_lowering=False, debug=False)
    xt = nc.declare_dram_parameter("xt", [W, nb], f16, isOutput=False)
    prm_d = nc.declare_dram_parameter("prm", [128, PRM_COLS], f32,
                                      isOutput=False)
    wts_d = nc.declare_dram_parameter("wts", [128, len(lay) * 128], bf16,
                                      isOutput=False)
    out_d = nc.declare_dram_parameter("out", [W, nb], f16, isOutput=True)

    nchunk = nb // ch          # chunks of 2048 cols (4 per core)

    with TileContext(nc) as tc:
        with (
            tc.tile_pool(name="const", bufs=1) as cpool,
            tc.tile_pool(name="v", bufs=3) as vpool,
            tc.tile_pool(name="t", bufs=3) as tpool,
            tc.tile_pool(name="q", bufs=3) as qpool,
            tc.tile_pool(name="s", bufs=2) as spool,
            tc.tile_pool(name="h", bufs=2) as hpool,
            tc.tile_pool(name="ps", bufs=2, space="PSUM") as pspool,
        ):
            prm = cpool.tile([128, PRM_COLS], f32, tag="prm", name="prm")
            nc.sync.dma_start(out=prm, in_=prm_d[:, :])
            nw = len(lay) * 128
            wsb = cpool.tile([128, nw], bf16, tag="wts", name="wsb")
            for s4 in range(4):
                lo, hi = s4 * nw // 4, (s4 + 1) * nw // 4
                nc.sync.dma_start(out=wsb[:, lo:hi], in_=wts_d[:, lo:hi])

            def pcol(kind, l, g):
                k = _PKINDS.index(kind)
                c = (k * DEPTH + l) * NTILE + g
                return prm[:, c:c + 1]

            def pcolF(g):
                return prm[:, len(_PKINDS) * DEPTH * NTILE + g:
                           len(_PKINDS) * DEPTH * NTILE + g + 1]

            def lhs(l, g, kind):
                j = widx[(l, g, kind)]
                return wsb[:, j * 128:(j + 1) * 128]

            xt_r = xt.rearrange("(g p) b -> p g b", g=NTILE)
            out_r = out_d.rearrange("(g p) b -> p g b", g=NTILE)

            SUPER = 2              # chunk-streams in flight
            for cp in range(nchunk // SUPER):
                cs = list(range(cp * SUPER, (cp + 1) * SUPER))
                Vs, Ts = {}, {}
                for c in cs:
                    Vs[c] = {}
                    for g in range(NTILE):
                        xv = vpool.tile([128, ch], f16, tag=f"v{g}",
                                        name=f"x{g}")
                        nc.sync.dma_start(
                            out=xv, in_=xt_r[:, g, c * ch:(c + 1) * ch])
                        Vs[c][g] = xv
                    Ts[c] = {}

                for l in range(DEPTH):
                  delta = DELTAS[l]
                  for c in cs:
                    V, T = Vs[c], Ts[c]
                    Vn, Tn = {}, {}
                    for g in range(NTILE):
                        ps = pspool.tile([128, ch], f32, tag="ps", name="ps")
                        for h in range(0, ch, MMH):
                            sl = slice(h, h + MMH)
                            if l == 0:
                                gp = g ^ (delta >> 7)
                                nc.tensor.matmul(
                                    ps[:, sl], lhs(l, g, "Ds"), V[g][:, sl],
                                    start=True, stop=False)
                                nc.tensor.matmul(
                                    ps[:, sl], lhs(l, g, "Dp"), V[gp][:, sl],
                                    start=False, stop=True)
                            elif l in CROSS:
                                gp = g ^ (delta >> 7)
                                nc.tensor.matmul(
                                    ps[:, sl], lhs(l, g, "Dsv"), V[g][:, sl],
                                    start=True, stop=False)
                                nc.tensor.matmul(
                                    ps[:, sl], lhs(l, g, "Dpv"), V[gp][:, sl],
                                    start=False, stop=False)
                                nc.tensor.matmul(
                                    ps[:, sl], lhs(l, g, "Dst"), T[g][:, sl],
                                    start=False, stop=False)
                                nc.tensor.matmul(
                                    ps[:, sl], lhs(l, g, "Dpt"), T[gp][:, sl],
                                    start=False, stop=True)
                            else:
                                nc.tensor.matmul(
                                    ps[:, sl], lhs(l, g, "MV"), V[g][:, sl],
                                    start=True, stop=False)
                                nc.tensor.matmul(
                                    ps[:, sl], lhs(l, g, "MT"), T[g][:, sl],
                                    start=False, stop=True)
                        # evacuate: v = (n * rhat) + (-pb*rhat)   [ACT]
                        v = vpool.tile([128, ch], f16, tag=f"v{g}",
                                       name=f"v{g}")
                        nc.scalar.activation(v, ps, AT.Identity,
                                             bias=pcol("bB", l, g),
                                             scale=pcol("sR", l, g))
                        # q = v*v  [DVE, fp16 2x]
                        q = qpool.tile([128, ch], f16, tag="q", name="q")
                        nc.vector.tensor_tensor(q, v, v, OP.mult)
                        # t = sqrt(q + 1)
                        t = tpool.tile([128, ch], f16, tag=f"t{g}",
                                       name=f"t{g}")
                        if (l + g + 2 * c) % 4 == 0:
                            nc.scalar.activation(t, q, AT.Sqrt, bias=1.0,
                                                 scale=1.0)
                        else:
                            s = spool.tile([128, ch], f16, tag="s", name="s")
                            nc.vector.tensor_scalar(s, q, 1.0, None, OP.add)
                            hh = hpool.tile([128, ch], f16, tag="h", name="h")
                            nc.vector.tensor_scalar(
                                hh.bitcast(i16), s.bitcast(i16), 1, None,
                                OP.logical_shift_right)
                            nc.vector.tensor_scalar(
                                t.bitcast(i16), hh.bitcast(i16), M_MAGIC,
                                None, OP.add)
                        Vn[g] = v
                        Tn[g] = t
                    Vs[c] = Vn
                    Ts[c] = Tn

                for c in cs:
                    for g in range(NTILE):
                        # F = V + ccF*T ; host applies sqrt(ik_31) scale
                        f = vpool.tile([128, ch], f16, tag=f"v{g}",
                                       name=f"o{g}")
                        nc.vector.scalar_tensor_tensor(
                            f, Ts[c][g], pcolF(g), Vs[c][g],
                            OP.mult, OP.add)
                        nc.sync.dma_start(
                            out=out_r[:, g, c * ch:(c + 1) * ch], in_=f)
    nc.compile()
    return nc


_NC_CACHE = {}

TRACE = False
TRACE_KWARGS = {}
LAST_RESULTS = None


def _get_nc(nb, ch):
    key = (nb, ch)
    if key not in _NC_CACHE:
        _NC_CACHE[key] = build_nc(nb, ch)
    return _NC_CACHE[key]


def kernel(X, thetas, biases, slopes1, slopes2, curvatures):
    global LAST_RESULTS
    from concourse.bass_utils import run_bass_kernel_spmd

    X = np.asarray(X)
    layers, out_perm = host_precompute(
        np.asarray(thetas), np.asarray(biases), np.asarray(slopes1),
        np.asarray(slopes2), np.asarray(curvatures))
    prm = pack_params(layers)
    wts = pack_weights(layers)

    nc = _get_nc(NB, CH)
    in_maps = []
    for cid in range(NCORES):
        shard = np.ascontiguousarray(
            X[cid * NB:(cid + 1) * NB, :].T.astype(np.float16))
        in_maps.append({"xt": shard, "prm": prm, "wts": wts})

    res = run_bass_kernel_spmd(nc, in_maps, list(range(NCORES)),
                               trace=TRACE, **TRACE_KWARGS)
    LAST_RESULTS = res
    # host-side: y = sqrt(ik_31) * F, then undo the final permutation
    D31 = layers[DEPTH - 1]["D"][out_perm].astype(np.float32)[:, None]
    out = np.empty((BATCH, W), dtype=np.float32)
    for cid in range(NCORES):
        o = res.results[cid]["out"]          # [512, NB] fp16 physical order
        out[cid * NB:(cid + 1) * NB, :] = \
            (o[out_perm, :].astype(np.float32) * D31).T
    return out
